# revision 14
# baseline (speedup 1.0000x reference)
"""Multi-head attention (B=4, N=2048, C=1024, H=16, D=64) on 8 TRN2 NeuronCores.

Sharding: core c owns (batch b = c//2, sequence half = c%2) -> 1024 query
tokens, all 16 heads.  Each core computes Q/K/V for its OWN half only; K and V
for the partner half arrive via pairwise AllGathers (replica groups
[2b, 2b+1], rank order = m order on both cores).

Perf structure (vs the v1 baseline):
- Score matmuls are issued as concurrent 64-row PE tiles for head pairs
  (2h, 2h+1): lhs/rhs at base partitions 0 and 64 land in different PE row
  groups, so both heads' S^T chunks compute simultaneously at full array
  utilization (the 50%-util score MMs of v1 kept the HAM clock gate at
  K=4/8 for the whole attention phase).
- exp runs mostly on ScalarE (true exp, scale fused); a configurable subset
  of tiles runs on VectorE via a Schraudolph bit-trick (int16(A*s+B) viewed
  as bf16), freeing ScalarE from being the pipeline limiter.
- Softmax denominators come from a ones-column appended to V inside the PV
  matmul (stationary [128, 65]); reciprocals use the fast custom-DVE approx
  batched per head-pair (v1 spent 126us in 8-cycle/elem DVE reciprocals).
- V is computed for the own half only and allgathered (v1 recomputed the
  full-sequence V on every core).
- All matmuls bf16 (f32 PSUM accumulate).
"""

import numpy as np
import ml_dtypes

import concourse.bass as bass
import concourse.mybir as mybir
import concourse.tile as tile
from concourse import bacc
from concourse.bass_utils import run_bass_kernel_spmd

B, N, C = 4, 2048, 1024
H, D = 16, 64
SCALE = D ** -0.5
NCORES = 8
NQ = N // 2          # query tokens per core (own half)
M = N                # key/value tokens after gather

BF16 = mybir.dt.bfloat16
F32 = mybir.dt.float32
I16 = mybir.dt.int16

# Schraudolph exp in bf16-bit space: bits = round(A*s + B), s = raw score
# (SCALE folded into A).  Calibrated for round-to-nearest f32->int16.
SCHRA_A = SCALE * 128.0 / float(np.log(2.0))
SCHRA_B = 16256.0 - 6.75
# Reciprocal seed in bf16-bit space: r0_bits = RECIP_C - d_bits, then one
# bf16 Newton step r1 = 2*r0 - r0*(d*r0)  (max rel err ~1.2%, rms 0.35%).
RECIP_C = 32500.0


_CACHE = {}
LAST_RESULTS = None


def _build():
    nc = bacc.Bacc(
        "TRN2",
        target_bir_lowering=False,
        debug=False,
        enable_asserts=False,
        num_devices=NCORES,
    )
    xT = nc.dram_tensor("xT", [C, M], BF16, kind="ExternalInput")
    xoT = nc.dram_tensor("xoT", [C, NQ], BF16, kind="ExternalInput")
    wqkvT = nc.dram_tensor("wqkvT", [1025, 3 * C], BF16, kind="ExternalInput")
    bqk = nc.dram_tensor("bqk", [2 * C, 1], F32, kind="ExternalInput")
    wprojT = nc.dram_tensor("wprojT", [C, C], BF16, kind="ExternalInput")
    bproj = nc.dram_tensor("bproj", [C, 1], F32, kind="ExternalInput")
    yT = nc.dram_tensor("yT", [C, NQ], F32, kind="ExternalOutput")

    groups = [[2 * b, 2 * b + 1] for b in range(B)]

    with tile.TileContext(nc) as tc:
        with (
            tc.tile_pool(name="persist", bufs=1) as pp,
            tc.tile_pool(name="dram", bufs=1, space="DRAM") as dp,
        ):
            lp = tc.alloc_tile_pool(name="qkv_in", bufs=1)
            psq = tc.alloc_tile_pool(name="psum_qkv", bufs=1, space="PSUM")
            xo_sb = []
            x_sb = []
            wq_sb = []
            for ct in range(8):
                xo_sb.append(lp.tile([128, NQ], BF16, tag=f"xo{ct}", name=f"xo{ct}"))
                x_sb.append(lp.tile([128, M], BF16, tag=f"x{ct}", name=f"x{ct}"))
                wq_sb.append(lp.tile([128, 3 * C], BF16, tag=f"wq{ct}", name=f"wq{ct}"))
            wqb = lp.tile([1, 3 * C], BF16, tag="wqb", name="wqb")
            # K columns first so the K matmuls (and the K AllGather) start as
            # early as possible; xT (for the full-sequence V) streams last.
            for ct in range(8):
                nc.sync.dma_start(xo_sb[ct][:, :], xoT[ct * 128 : (ct + 1) * 128, :])
                nc.sync.dma_start(wq_sb[ct][:, C : 2 * C], wqkvT[ct * 128 : (ct + 1) * 128, C : 2 * C])
            for ct in range(8):
                nc.sync.dma_start(wq_sb[ct][:, 0:C], wqkvT[ct * 128 : (ct + 1) * 128, 0:C])
                nc.sync.dma_start(wq_sb[ct][:, 2 * C :], wqkvT[ct * 128 : (ct + 1) * 128, 2 * C :])
            nc.sync.dma_start(wqb[:, :], wqkvT[1024:1025, :])
            for ct in range(8):
                nc.sync.dma_start(x_sb[ct][:, :], xT[ct * 128 : (ct + 1) * 128, :])

            bp_sb = []
            bq_sb = []
            bk_sb = []
            for i in range(8):
                t = pp.tile([128, 1], F32, tag=f"bp{i}", name=f"bp{i}")
                nc.sync.dma_start(t[:, :], bproj[i * 128 : (i + 1) * 128, :])
                bp_sb.append(t)
                t = pp.tile([128, 1], F32, tag=f"bq{i}", name=f"bq{i}")
                nc.sync.dma_start(t[:, :], bqk[i * 128 : (i + 1) * 128, :])
                bq_sb.append(t)
                t = pp.tile([128, 1], F32, tag=f"bk{i}", name=f"bk{i}")
                nc.sync.dma_start(t[:, :], bqk[C + i * 128 : C + (i + 1) * 128, :])
                bk_sb.append(t)

            QT_sb = [pp.tile([128, NQ], BF16, tag=f"qt{i}", name=f"qt{i}") for i in range(8)]
            KT_sb = [pp.tile([128, M], BF16, tag=f"kt{i}", name=f"kt{i}") for i in range(8)]
            V_sb = [pp.tile([128, H, D + 1], BF16, tag=f"v{mt}", name=f"v{mt}") for mt in range(16)]
            A_sb = [pp.tile([128, NQ], BF16, tag=f"a{i}", name=f"a{i}") for i in range(8)]

            # DRAM bounce buffers for the pairwise K/V AllGathers (2 chunks each)
            k_in = [dp.tile([512, NQ], BF16, tag=f"ki{c}", name=f"ki{c}") for c in range(2)]
            k_out = [dp.tile([2, 512, NQ], BF16, tag=f"ko{c}", name=f"ko{c}") for c in range(2)]

            # V bias broadcast tile (from the wqkv bias row)
            bvb = lp.tile([128, C], BF16, tag="bvb", name="bvb")
            nc.gpsimd.partition_broadcast(bvb[:, :], wqb[0:1, 2 * C :])

            # PE warmup: ~40 back-to-back matmuls on scratch so the HAM clock
            # gate reaches K=8/8 while the input DMAs stream in; also preload
            # the exp activation table (Identity shares its set).
            wu_s = lp.tile([128, 512], BF16, tag="wu_s", name="wu_s")
            nc.vector.memset(wu_s[:, :], 0.125)
            pre_t = lp.tile([1, 16], BF16, tag="pre_t", name="pre_t")
            nc.scalar.activation(
                pre_t[:, :], wu_s[0:1, 0:16],
                mybir.ActivationFunctionType.Exp,
            )
            wu_p = psq.tile([128, 512], F32, tag="wu", bufs=1, name="wu_p")
            for _ in range(64):
                nc.tensor.matmul(
                    wu_p[:, :], wu_s[:, 0:128], wu_s[:, :],
                    start=True, stop=True, skip_group_check=True,
                )

            for mt in range(16):
                nc.vector.memset(V_sb[mt][:, :, D : D + 1], 1.0)

            # ---- K own-half (bias fused into the DVE drain), bounced via
            # DRAM for the AllGather.  Both ranks' halves are DMA'd back from
            # k_out (rank order = m order, identical on both cores of a pair).
            for i in range(8):
                c = i // 4
                ps = psq.tile([128, NQ], F32, tag="mm", bufs=2, name="psk")
                for ct in range(8):
                    for nch2 in range(2):
                        nc.tensor.matmul(
                            ps[:, nch2 * 512 : (nch2 + 1) * 512],
                            wq_sb[ct][:, C + i * 128 : C + (i + 1) * 128],
                            xo_sb[ct][:, nch2 * 512 : (nch2 + 1) * 512],
                            start=(ct == 0),
                            stop=(ct == 7),
                        )
                kh = lp.tile([128, NQ], BF16, tag="kh", bufs=2, name="kh")
                nc.vector.tensor_scalar_add(kh[:, :], ps[:, :], bk_sb[i][:, :])
                nc.sync.dma_start(k_in[c][(i % 4) * 128 : (i % 4 + 1) * 128, :], kh[:, :])
                if i % 4 == 3:
                    nc.gpsimd.collective_compute(
                        "AllGather",
                        mybir.AluOpType.bypass,
                        replica_groups=groups,
                        ins=[k_in[c].opt()],
                        outs=[k_out[c].opt()],
                    )

            # ---- V for the FULL sequence (local; bias via broadcast add)
            for mt in range(16):
                ps = psq.tile([128, H, D], F32, tag="mm", bufs=2, name="psv")
                for ct in range(8):
                    for vch in range(2):
                        nc.tensor.matmul(
                            ps[:, vch * 8 : (vch + 1) * 8, :],
                            x_sb[ct][:, mt * 128 : (mt + 1) * 128],
                            wq_sb[ct][:, 2 * C + vch * 512 : 2 * C + (vch + 1) * 512],
                            start=(ct == 0),
                            stop=(ct == 7),
                        )
                nc.vector.tensor_tensor(
                    V_sb[mt][:, :, 0:D], ps[:, :, :],
                    bvb[:, :].rearrange("p (h e) -> p h e", e=D),
                    op=mybir.AluOpType.add,
                )

            # ---- gathered K -> SBUF (both ranks; rank order = m order)
            for c in range(2):
                for r in range(2):
                    for ii in range(4):
                        i = c * 4 + ii
                        nc.sync.dma_start(
                            KT_sb[i][:, r * NQ : (r + 1) * NQ],
                            k_out[c][r, ii * 128 : (ii + 1) * 128, :],
                        )

            # ---- Q (bias fused into the ScalarE drain)
            for i in range(8):
                ps = psq.tile([128, NQ], F32, tag="mm", bufs=2, name="psq")
                for ct in range(8):
                    for nch2 in range(2):
                        nc.tensor.matmul(
                            ps[:, nch2 * 512 : (nch2 + 1) * 512],
                            wq_sb[ct][:, i * 128 : (i + 1) * 128],
                            xo_sb[ct][:, nch2 * 512 : (nch2 + 1) * 512],
                            start=(ct == 0),
                            stop=(ct == 7),
                        )
                nc.scalar.activation(
                    QT_sb[i][:, :], ps[:, :],
                    mybir.ActivationFunctionType.Identity,
                    bias=bq_sb[i][:, :],
                )
            lp.release()
            psq.release()

            # ---- attention: head pairs (2i, 2i+1) at PE row groups 0/64 ----
            psa = tc.alloc_tile_pool(name="psum_attn", bufs=1, space="PSUM")
            wk = tc.alloc_tile_pool(name="attnwork", bufs=1)
            wp_sb = []
            for i in range(8):
                t = wk.tile([128, C], BF16, tag=f"wp{i}", name=f"wp{i}")
                nc.sync.dma_start(t[:, :], wprojT[i * 128 : (i + 1) * 128, :])
                wp_sb.append(t)

            tail_ops = []

            def enqueue_norm(i, stA, stB, dA, dB):
                # 1/den Newton chain (bit-trick seed + one bf16 step).  Each
                # step is queued as a closure and drained one-per-mt inside
                # the NEXT pair's loop so the DVE never sees a burst that
                # would stall its exp stream (a burst cold-dips the PE).
                for j, st, d0 in ((0, stA, dA), (1, stB, dB)):
                    box = {}

                    def mk(fn):
                        tail_ops.append(fn)

                    def op_seed(j=j, d0=d0, box=box):
                        r0 = wk.tile([1, NQ], BF16, tag="r0", bufs=2, name="r0")
                        nc.vector.tensor_scalar(
                            r0[:, :].bitcast(I16), d0[:, :].bitcast(I16),
                            -1.0, RECIP_C,
                            mybir.AluOpType.mult, mybir.AluOpType.add,
                        )
                        box["r0"] = r0
                    mk(op_seed)

                    def op_t(j=j, d0=d0, box=box):
                        t = wk.tile([1, NQ], BF16, tag="t", bufs=2, name="t")
                        nc.vector.tensor_mul(t[:, :], d0[:, :], box["r0"][:, :])
                        box["t"] = t
                    mk(op_t)

                    def op_u(box=box):
                        u = wk.tile([1, NQ], BF16, tag="u", bufs=2, name="u")
                        nc.vector.tensor_mul(u[:, :], box["r0"][:, :], box["t"][:, :])
                        box["u"] = u
                    mk(op_u)

                    def op_r1(box=box):
                        r1 = wk.tile([1, NQ], BF16, tag="r1", bufs=2, name="r1")
                        nc.vector.scalar_tensor_tensor(
                            r1[:, :], box["r0"][:, :], 2.0, box["u"][:, :],
                            mybir.AluOpType.mult, mybir.AluOpType.subtract,
                        )
                        box["r1"] = r1
                    mk(op_r1)

                    def op_bcast(box=box):
                        rb = wk.tile([64, NQ], BF16, tag="rb", bufs=2, name="rb")
                        nc.gpsimd.partition_broadcast(rb[:, :], box["r1"][0:1, :])
                        box["rb"] = rb
                    mk(op_bcast)

                    def op_mul(i=i, j=j, st=st, box=box):
                        nc.vector.tensor_mul(
                            A_sb[i][j * 64 : (j + 1) * 64, :], st[0:64, :],
                            box["rb"][:, :],
                        )
                    mk(op_mul)

            for i in range(8):
                hA, hB = 2 * i, 2 * i + 1
                pvA = psa.tile([128, NQ], F32, tag="pvA", bufs=1, name="pvA")
                pvB = psa.tile([128, NQ], F32, tag="pvB", bufs=1, name="pvB")
                for mt in range(16):
                    mtc = slice(mt * 128, (mt + 1) * 128)
                    pA = wk.tile([128, NQ], BF16, tag="p", bufs=6, name="pA")
                    pB = wk.tile([128, NQ], BF16, tag="p", bufs=6, name="pB")
                    sAs, sBs = [], []
                    for nch in range(2):
                        ncs = slice(nch * 512, (nch + 1) * 512)
                        # [128, 512] score tiles, double-buffered: scores for
                        # mt+1 never wait on exp of mt, so the PE streams the
                        # row-group pair concurrently and stays HAM-warm.
                        sA = psa.tile([128, 512], F32, tag="sA", bufs=2, name="sA")
                        sB = psa.tile([128, 512], F32, tag="sB", bufs=2, name="sB")
                        nc.tensor.matmul(
                            sA[:, :], KT_sb[i][0:64, mtc], QT_sb[i][0:64, ncs],
                            start=True, stop=True,
                        )
                        nc.tensor.matmul(
                            sB[:, :], KT_sb[i][64:128, mtc], QT_sb[i][64:128, ncs],
                            start=True, stop=True,
                        )
                        sAs.append(sA)
                        sBs.append(sB)
                    for nch in range(2):
                        ncs = slice(nch * 512, (nch + 1) * 512)
                        # head A: true exp on ScalarE; head B: Schraudolph
                        # exp on VectorE (int16 bits of bf16); concurrent
                        # engines so neither paces the PE.
                        nc.scalar.activation(
                            pA[:, ncs], sAs[nch][:, :],
                            mybir.ActivationFunctionType.Exp, scale=SCALE,
                        )
                        nc.vector.tensor_scalar(
                            pB[:, ncs].bitcast(I16), sBs[nch][:, :],
                            SCHRA_A, SCHRA_B,
                            mybir.AluOpType.mult,
                            mybir.AluOpType.add,
                        )
                    for nch in range(2):
                        ncs = slice(nch * 512, (nch + 1) * 512)
                        nc.tensor.matmul(
                            pvA[0:65, ncs], V_sb[mt][:, hA, :], pA[:, ncs],
                            start=(mt == 0), stop=(mt == 15),
                            skip_group_check=True,
                        )
                        nc.tensor.matmul(
                            pvB[0:65, ncs], V_sb[mt][:, hB, :], pB[:, ncs],
                            start=(mt == 0), stop=(mt == 15),
                            skip_group_check=True,
                        )
                # stage PV+den to SBUF (split across ScalarE/VectorE; PSUM
                # banks recycle for pair i+1); dens land in base-0 tiles
                stA = wk.tile([65, NQ], BF16, tag="st", bufs=4, name="stA")
                stB = wk.tile([65, NQ], BF16, tag="st", bufs=4, name="stB")
                dA = wk.tile([1, NQ], BF16, tag="dd", bufs=4, name="dA")
                dB = wk.tile([1, NQ], BF16, tag="dd", bufs=4, name="dB")
                nc.scalar.copy(stA[:, :], pvA[0:65, :])
                nc.vector.tensor_copy(stB[:, :], pvB[0:65, :])
                nc.scalar.copy(dA[:, :], pvA[64:65, :])
                nc.vector.tensor_copy(dB[:, :], pvB[64:65, :])
                prev = len(tail_ops)
                enqueue_norm(i, stA, stB, dA, dB)
                if i >= 1:
                    for _ in range(prev):
                        tail_ops.pop(0)()
            while tail_ops:
                tail_ops.pop(0)()
            psa.release()

            # ---- output projection (pairs of output tiles: 4 accumulators) ----
            psp2 = tc.alloc_tile_pool(name="psum_proj", bufs=1, space="PSUM")
            for op2 in range(4):
                pss = [
                    psp2.tile([128, 512], F32, tag=f"acc{j}_{nch}", bufs=1, name="psp")
                    for j in range(2)
                    for nch in range(2)
                ]
                for dd in range(8):
                    for j in range(2):
                        ot = op2 * 2 + j
                        for nch in range(2):
                            nc.tensor.matmul(
                                pss[j * 2 + nch][:, :],
                                wp_sb[dd][:, ot * 128 : (ot + 1) * 128],
                                A_sb[dd][:, nch * 512 : (nch + 1) * 512],
                                start=(dd == 0),
                                stop=(dd == 7),
                            )
                for j in range(2):
                    ot = op2 * 2 + j
                    for nch in range(2):
                        y = wk.tile([128, 512], F32, tag="y", bufs=3, name="y")
                        nc.scalar.activation(
                            y[:, :], pss[j * 2 + nch][:, :],
                            mybir.ActivationFunctionType.Identity,
                            bias=bp_sb[ot][:, :],
                        )
                        nc.sync.dma_start(
                            yT[ot * 128 : (ot + 1) * 128, nch * 512 : (nch + 1) * 512],
                            y[:, :],
                        )
            wk.release()
            psp2.release()

    nc.compile()
    return nc


def kernel(x, w_qkv, b_qkv, w_proj, b_proj):
    global LAST_RESULTS
    bf = ml_dtypes.bfloat16
    x = np.asarray(x, np.float32)
    w_qkv = np.asarray(w_qkv, np.float32)
    b_qkv = np.asarray(b_qkv, np.float32)
    w_proj = np.asarray(w_proj, np.float32)
    b_proj = np.asarray(b_proj, np.float32)

    wqkvT = np.ascontiguousarray(
        np.vstack([w_qkv.T, b_qkv[None, :]]).astype(bf)
    )  # [1025, 3072]
    wprojT = np.ascontiguousarray(w_proj.T.astype(bf))  # [1024, 1024]
    bqk = np.ascontiguousarray(b_qkv[: 2 * C, None].astype(np.float32))  # [2048, 1]
    bproj = np.ascontiguousarray(b_proj[:, None].astype(np.float32))  # [1024, 1]

    in_maps = []
    xTb = {b: np.ascontiguousarray(x[b].T.astype(bf)) for b in range(B)}
    for core in range(NCORES):
        b, half = core // 2, core % 2
        own = x[b][half * NQ : (half + 1) * NQ]  # [1024, 1024]
        in_maps.append(
            {
                "xT": xTb[b],
                "xoT": np.ascontiguousarray(own.T.astype(bf)),
                "wqkvT": wqkvT,
                "bqk": bqk,
                "wprojT": wprojT,
                "bproj": bproj,
            }
        )

    if "nc" not in _CACHE:
        _CACHE["nc"] = _build()
    nc = _CACHE["nc"]

    res = run_bass_kernel_spmd(nc, in_maps, core_ids=list(range(NCORES)))
    LAST_RESULTS = res

    out = np.empty((B, N, C), np.float32)
    for core in range(NCORES):
        b, half = core // 2, core % 2
        out[b, half * NQ : (half + 1) * NQ, :] = res.results[core]["yT"].T
    return out


if __name__ == "__main__":
    rng = np.random.default_rng(0)
    s = C ** -0.5
    ins = {
        "x": rng.standard_normal((B, N, C)).astype(np.float32),
        "w_qkv": (rng.standard_normal((3 * C, C)) * s).astype(np.float32),
        "b_qkv": (rng.standard_normal(3 * C) * 0.02).astype(np.float32),
        "w_proj": (rng.standard_normal((C, C)) * s).astype(np.float32),
        "b_proj": (rng.standard_normal(C) * 0.02).astype(np.float32),
    }
    y = kernel(**ins)
    print("out", y.shape, y.dtype, float(np.abs(y).mean()))


# revision 15
# speedup vs baseline: 1.0924x; 1.0924x over previous
"""Multi-head attention (B=4, N=2048, C=1024, H=16, D=64) on 8 TRN2 NeuronCores.

Sharding: core c owns (batch b = c//2, sequence half = c%2) -> 1024 query
tokens, all 16 heads.  Each core computes Q/K/V for its OWN half only; K and V
for the partner half arrive via pairwise AllGathers (replica groups
[2b, 2b+1], rank order = m order on both cores).

Perf structure (vs the v1 baseline):
- Score matmuls are issued as concurrent 64-row PE tiles for head pairs
  (2h, 2h+1): lhs/rhs at base partitions 0 and 64 land in different PE row
  groups, so both heads' S^T chunks compute simultaneously at full array
  utilization (the 50%-util score MMs of v1 kept the HAM clock gate at
  K=4/8 for the whole attention phase).
- exp runs mostly on ScalarE (true exp, scale fused); a configurable subset
  of tiles runs on VectorE via a Schraudolph bit-trick (int16(A*s+B) viewed
  as bf16), freeing ScalarE from being the pipeline limiter.
- Softmax denominators come from a ones-column appended to V inside the PV
  matmul (stationary [128, 65]); reciprocals use the fast custom-DVE approx
  batched per head-pair (v1 spent 126us in 8-cycle/elem DVE reciprocals).
- V is computed for the own half only and allgathered (v1 recomputed the
  full-sequence V on every core).
- All matmuls bf16 (f32 PSUM accumulate).
"""

import numpy as np
import ml_dtypes

import concourse.bass as bass
import concourse.mybir as mybir
import concourse.tile as tile
from concourse import bacc
from concourse.bass_utils import run_bass_kernel_spmd

B, N, C = 4, 2048, 1024
H, D = 16, 64
SCALE = D ** -0.5
NCORES = 8
NQ = N // 2          # query tokens per core (own half)
M = N                # key/value tokens after gather

BF16 = mybir.dt.bfloat16
F32 = mybir.dt.float32
I16 = mybir.dt.int16

# Schraudolph exp in bf16-bit space: bits = round(A*s + B), s = raw score
# (SCALE folded into A).  Calibrated for round-to-nearest f32->int16.
SCHRA_A = SCALE * 128.0 / float(np.log(2.0))
SCHRA_B = 16256.0 - 6.75
# Reciprocal seed in bf16-bit space: r0_bits = RECIP_C - d_bits, then one
# bf16 Newton step r1 = 2*r0 - r0*(d*r0)  (max rel err ~1.2%, rms 0.35%).
RECIP_C = 32500.0


_CACHE = {}
LAST_RESULTS = None


def _build():
    nc = bacc.Bacc(
        "TRN2",
        target_bir_lowering=False,
        debug=False,
        enable_asserts=False,
        num_devices=NCORES,
    )
    xT = nc.dram_tensor("xT", [C, M], BF16, kind="ExternalInput")
    xoT = nc.dram_tensor("xoT", [C, NQ], BF16, kind="ExternalInput")
    wqkvT = nc.dram_tensor("wqkvT", [1025, 3 * C], BF16, kind="ExternalInput")
    bqk = nc.dram_tensor("bqk", [2 * C, 1], F32, kind="ExternalInput")
    wprojT = nc.dram_tensor("wprojT", [C, C], BF16, kind="ExternalInput")
    bproj = nc.dram_tensor("bproj", [C, 1], F32, kind="ExternalInput")
    yT = nc.dram_tensor("yT", [C, NQ], F32, kind="ExternalOutput")

    groups = [[2 * b, 2 * b + 1] for b in range(B)]

    with tile.TileContext(nc) as tc:
        with (
            tc.tile_pool(name="persist", bufs=1) as pp,
            tc.tile_pool(name="dram", bufs=1, space="DRAM") as dp,
        ):
            lp = tc.alloc_tile_pool(name="qkv_in", bufs=1)
            psq = tc.alloc_tile_pool(name="psum_qkv", bufs=1, space="PSUM")
            xo_sb = []
            x_sb = []
            wq_sb = []
            for ct in range(8):
                xo_sb.append(lp.tile([128, NQ], BF16, tag=f"xo{ct}", name=f"xo{ct}"))
                x_sb.append(lp.tile([128, M], BF16, tag=f"x{ct}", name=f"x{ct}"))
                wq_sb.append(lp.tile([128, 3 * C], BF16, tag=f"wq{ct}", name=f"wq{ct}"))
            wqb = lp.tile([1, 3 * C], BF16, tag="wqb", name="wqb")
            # K columns first so the K matmuls (and the K AllGather) start as
            # early as possible; xT (for the full-sequence V) streams last.
            for ct in range(8):
                nc.sync.dma_start(xo_sb[ct][:, :], xoT[ct * 128 : (ct + 1) * 128, :])
                nc.sync.dma_start(wq_sb[ct][:, C : 2 * C], wqkvT[ct * 128 : (ct + 1) * 128, C : 2 * C])
            for ct in range(8):
                nc.sync.dma_start(wq_sb[ct][:, 0:C], wqkvT[ct * 128 : (ct + 1) * 128, 0:C])
                nc.sync.dma_start(wq_sb[ct][:, 2 * C :], wqkvT[ct * 128 : (ct + 1) * 128, 2 * C :])
            nc.sync.dma_start(wqb[:, :], wqkvT[1024:1025, :])
            for ct in range(8):
                nc.sync.dma_start(x_sb[ct][:, :], xT[ct * 128 : (ct + 1) * 128, :])

            bp_sb = []
            bq_sb = []
            bk_sb = []
            for i in range(8):
                t = pp.tile([128, 1], F32, tag=f"bp{i}", name=f"bp{i}")
                nc.sync.dma_start(t[:, :], bproj[i * 128 : (i + 1) * 128, :])
                bp_sb.append(t)
                t = pp.tile([128, 1], F32, tag=f"bq{i}", name=f"bq{i}")
                nc.sync.dma_start(t[:, :], bqk[i * 128 : (i + 1) * 128, :])
                bq_sb.append(t)
                t = pp.tile([128, 1], F32, tag=f"bk{i}", name=f"bk{i}")
                nc.sync.dma_start(t[:, :], bqk[C + i * 128 : C + (i + 1) * 128, :])
                bk_sb.append(t)

            QT_sb = [pp.tile([128, NQ], BF16, tag=f"qt{i}", name=f"qt{i}") for i in range(8)]
            KT_sb = [pp.tile([128, M], BF16, tag=f"kt{i}", name=f"kt{i}") for i in range(8)]
            V_sb = [pp.tile([128, H, D + 1], BF16, tag=f"v{mt}", name=f"v{mt}") for mt in range(16)]
            A_sb = [pp.tile([128, NQ], BF16, tag=f"a{i}", name=f"a{i}") for i in range(8)]

            # DRAM bounce buffers for the pairwise K/V AllGathers (2 chunks each)
            k_in = [dp.tile([512, NQ], BF16, tag=f"ki{c}", name=f"ki{c}") for c in range(2)]
            k_out = [dp.tile([2, 512, NQ], BF16, tag=f"ko{c}", name=f"ko{c}") for c in range(2)]

            # V bias broadcast tile (from the wqkv bias row)
            bvb = lp.tile([128, C], BF16, tag="bvb", name="bvb")
            nc.gpsimd.partition_broadcast(bvb[:, :], wqb[0:1, 2 * C :])

            # PE warmup: ~40 back-to-back matmuls on scratch so the HAM clock
            # gate reaches K=8/8 while the input DMAs stream in; also preload
            # the exp activation table (Identity shares its set).
            wu_s = lp.tile([128, 512], BF16, tag="wu_s", name="wu_s")
            nc.vector.memset(wu_s[:, :], 0.125)
            pre_t = lp.tile([1, 16], BF16, tag="pre_t", name="pre_t")
            nc.scalar.activation(
                pre_t[:, :], wu_s[0:1, 0:16],
                mybir.ActivationFunctionType.Exp,
            )
            wu_p = psq.tile([128, 512], F32, tag="wu", bufs=1, name="wu_p")
            for _ in range(40):
                nc.tensor.matmul(
                    wu_p[:, :], wu_s[:, 0:128], wu_s[:, :],
                    start=True, stop=True, skip_group_check=True,
                )

            for mt in range(16):
                nc.vector.memset(V_sb[mt][:, :, D : D + 1], 1.0)

            # ---- K own-half (bias fused into the DVE drain), bounced via
            # DRAM for the AllGather.  Both ranks' halves are DMA'd back from
            # k_out (rank order = m order, identical on both cores of a pair).
            for i in range(8):
                c = i // 4
                ps = psq.tile([128, NQ], F32, tag="mm", bufs=2, name="psk")
                for ct in range(8):
                    for nch2 in range(2):
                        nc.tensor.matmul(
                            ps[:, nch2 * 512 : (nch2 + 1) * 512],
                            wq_sb[ct][:, C + i * 128 : C + (i + 1) * 128],
                            xo_sb[ct][:, nch2 * 512 : (nch2 + 1) * 512],
                            start=(ct == 0),
                            stop=(ct == 7),
                        )
                kh = lp.tile([128, NQ], BF16, tag="kh", bufs=2, name="kh")
                nc.vector.tensor_scalar_add(kh[:, :], ps[:, :], bk_sb[i][:, :])
                nc.sync.dma_start(k_in[c][(i % 4) * 128 : (i % 4 + 1) * 128, :], kh[:, :])
                if i % 4 == 3:
                    nc.gpsimd.collective_compute(
                        "AllGather",
                        mybir.AluOpType.bypass,
                        replica_groups=groups,
                        ins=[k_in[c].opt()],
                        outs=[k_out[c].opt()],
                    )

            # ---- V for the FULL sequence (local; bias via broadcast add)
            for mt in range(16):
                ps = psq.tile([128, H, D], F32, tag="mm", bufs=2, name="psv")
                for ct in range(8):
                    for vch in range(2):
                        nc.tensor.matmul(
                            ps[:, vch * 8 : (vch + 1) * 8, :],
                            x_sb[ct][:, mt * 128 : (mt + 1) * 128],
                            wq_sb[ct][:, 2 * C + vch * 512 : 2 * C + (vch + 1) * 512],
                            start=(ct == 0),
                            stop=(ct == 7),
                        )
                nc.vector.tensor_tensor(
                    V_sb[mt][:, :, 0:D], ps[:, :, :],
                    bvb[:, :].rearrange("p (h e) -> p h e", e=D),
                    op=mybir.AluOpType.add,
                )

            # ---- gathered K -> SBUF (both ranks; rank order = m order)
            for c in range(2):
                for r in range(2):
                    for ii in range(4):
                        i = c * 4 + ii
                        nc.sync.dma_start(
                            KT_sb[i][:, r * NQ : (r + 1) * NQ],
                            k_out[c][r, ii * 128 : (ii + 1) * 128, :],
                        )

            # ---- Q (bias fused into the ScalarE drain)
            for i in range(8):
                ps = psq.tile([128, NQ], F32, tag="mm", bufs=2, name="psq")
                for ct in range(8):
                    for nch2 in range(2):
                        nc.tensor.matmul(
                            ps[:, nch2 * 512 : (nch2 + 1) * 512],
                            wq_sb[ct][:, i * 128 : (i + 1) * 128],
                            xo_sb[ct][:, nch2 * 512 : (nch2 + 1) * 512],
                            start=(ct == 0),
                            stop=(ct == 7),
                        )
                nc.scalar.activation(
                    QT_sb[i][:, :], ps[:, :],
                    mybir.ActivationFunctionType.Identity,
                    bias=bq_sb[i][:, :],
                )
            lp.release()
            psq.release()

            # ---- attention: head pairs (2i, 2i+1) at PE row groups 0/64 ----
            psa = tc.alloc_tile_pool(name="psum_attn", bufs=1, space="PSUM")
            wk = tc.alloc_tile_pool(name="attnwork", bufs=1)
            wp_sb = []
            for i in range(8):
                t = wk.tile([128, C], BF16, tag=f"wp{i}", name=f"wp{i}")
                nc.sync.dma_start(t[:, :], wprojT[i * 128 : (i + 1) * 128, :])
                wp_sb.append(t)

            tail_ops = []

            def enqueue_norm(i, stA, stB, dA, dB):
                # 1/den Newton chain (bit-trick seed + one bf16 step).  Each
                # step is queued as a closure and drained one-per-mt inside
                # the NEXT pair's loop so the DVE never sees a burst that
                # would stall its exp stream (a burst cold-dips the PE).
                for j, st, d0 in ((0, stA, dA), (1, stB, dB)):
                    box = {}

                    def mk(fn):
                        tail_ops.append(fn)

                    def op_seed(j=j, d0=d0, box=box):
                        r0 = wk.tile([1, NQ], BF16, tag="r0", bufs=2, name="r0")
                        nc.vector.tensor_scalar(
                            r0[:, :].bitcast(I16), d0[:, :].bitcast(I16),
                            -1.0, RECIP_C,
                            mybir.AluOpType.mult, mybir.AluOpType.add,
                        )
                        box["r0"] = r0
                    mk(op_seed)

                    def op_t(j=j, d0=d0, box=box):
                        t = wk.tile([1, NQ], BF16, tag="t", bufs=2, name="t")
                        nc.vector.tensor_mul(t[:, :], d0[:, :], box["r0"][:, :])
                        box["t"] = t
                    mk(op_t)

                    def op_u(box=box):
                        u = wk.tile([1, NQ], BF16, tag="u", bufs=2, name="u")
                        nc.vector.tensor_mul(u[:, :], box["r0"][:, :], box["t"][:, :])
                        box["u"] = u
                    mk(op_u)

                    def op_r1(box=box):
                        r1 = wk.tile([1, NQ], BF16, tag="r1", bufs=2, name="r1")
                        nc.vector.scalar_tensor_tensor(
                            r1[:, :], box["r0"][:, :], 2.0, box["u"][:, :],
                            mybir.AluOpType.mult, mybir.AluOpType.subtract,
                        )
                        box["r1"] = r1
                    mk(op_r1)

                    def op_bcast(box=box):
                        rb = wk.tile([64, NQ], BF16, tag="rb", bufs=2, name="rb")
                        nc.gpsimd.partition_broadcast(rb[:, :], box["r1"][0:1, :])
                        box["rb"] = rb
                    mk(op_bcast)

                    def op_mul(i=i, j=j, st=st, box=box):
                        nc.vector.tensor_mul(
                            A_sb[i][j * 64 : (j + 1) * 64, :], st[0:64, :],
                            box["rb"][:, :],
                        )
                    mk(op_mul)

            for i in range(8):
                hA, hB = 2 * i, 2 * i + 1
                pvA = psa.tile([128, NQ], F32, tag="pvA", bufs=1, name="pvA")
                pvB = psa.tile([128, NQ], F32, tag="pvB", bufs=1, name="pvB")
                for mt in range(16):
                    mtc = slice(mt * 128, (mt + 1) * 128)
                    pA = wk.tile([128, NQ], BF16, tag="p", bufs=6, name="pA")
                    pB = wk.tile([128, NQ], BF16, tag="p", bufs=6, name="pB")
                    sAs, sBs = [], []
                    for nch in range(2):
                        ncs = slice(nch * 512, (nch + 1) * 512)
                        # [128, 512] score tiles, double-buffered: scores for
                        # mt+1 never wait on exp of mt, so the PE streams the
                        # row-group pair concurrently and stays HAM-warm.
                        sA = psa.tile([128, 512], F32, tag="sA", bufs=2, name="sA")
                        sB = psa.tile([128, 512], F32, tag="sB", bufs=2, name="sB")
                        nc.tensor.matmul(
                            sA[:, :], KT_sb[i][0:64, mtc], QT_sb[i][0:64, ncs],
                            start=True, stop=True,
                        )
                        nc.tensor.matmul(
                            sB[:, :], KT_sb[i][64:128, mtc], QT_sb[i][64:128, ncs],
                            start=True, stop=True,
                        )
                        sAs.append(sA)
                        sBs.append(sB)
                    for nch in range(2):
                        ncs = slice(nch * 512, (nch + 1) * 512)
                        # head A: true exp on ScalarE; head B: Schraudolph
                        # exp on VectorE (int16 bits of bf16); concurrent
                        # engines so neither paces the PE.
                        nc.scalar.activation(
                            pA[:, ncs], sAs[nch][:, :],
                            mybir.ActivationFunctionType.Exp, scale=SCALE,
                        )
                        if nch == 1 and mt % 2 == 1:
                            nc.scalar.activation(
                                pB[:, ncs], sBs[nch][:, :],
                                mybir.ActivationFunctionType.Exp, scale=SCALE,
                            )
                        else:
                            nc.vector.tensor_scalar(
                                pB[:, ncs].bitcast(I16), sBs[nch][:, :],
                                SCHRA_A, SCHRA_B,
                                mybir.AluOpType.mult,
                                mybir.AluOpType.add,
                            )
                    for nch in range(2):
                        ncs = slice(nch * 512, (nch + 1) * 512)
                        nc.tensor.matmul(
                            pvA[0:65, ncs], V_sb[mt][:, hA, :], pA[:, ncs],
                            start=(mt == 0), stop=(mt == 15),
                            skip_group_check=True,
                        )
                        nc.tensor.matmul(
                            pvB[0:65, ncs], V_sb[mt][:, hB, :], pB[:, ncs],
                            start=(mt == 0), stop=(mt == 15),
                            skip_group_check=True,
                        )
                # stage PV+den to SBUF (split across ScalarE/VectorE; PSUM
                # banks recycle for pair i+1); dens land in base-0 tiles
                stA = wk.tile([65, NQ], BF16, tag="st", bufs=4, name="stA")
                stB = wk.tile([65, NQ], BF16, tag="st", bufs=4, name="stB")
                dA = wk.tile([1, NQ], BF16, tag="dd", bufs=4, name="dA")
                dB = wk.tile([1, NQ], BF16, tag="dd", bufs=4, name="dB")
                nc.scalar.copy(stA[:, :], pvA[0:65, :])
                nc.vector.tensor_copy(stB[:, :], pvB[0:65, :])
                nc.scalar.copy(dA[:, :], pvA[64:65, :])
                nc.vector.tensor_copy(dB[:, :], pvB[64:65, :])
                prev = len(tail_ops)
                enqueue_norm(i, stA, stB, dA, dB)
                if i >= 1:
                    for _ in range(prev):
                        tail_ops.pop(0)()
            while tail_ops:
                tail_ops.pop(0)()
            psa.release()

            # ---- output projection (pairs of output tiles: 4 accumulators) ----
            psp2 = tc.alloc_tile_pool(name="psum_proj", bufs=1, space="PSUM")
            for op2 in range(4):
                pss = [
                    psp2.tile([128, 512], F32, tag=f"acc{j}_{nch}", bufs=1, name="psp")
                    for j in range(2)
                    for nch in range(2)
                ]
                for dd in range(8):
                    for j in range(2):
                        ot = op2 * 2 + j
                        for nch in range(2):
                            nc.tensor.matmul(
                                pss[j * 2 + nch][:, :],
                                wp_sb[dd][:, ot * 128 : (ot + 1) * 128],
                                A_sb[dd][:, nch * 512 : (nch + 1) * 512],
                                start=(dd == 0),
                                stop=(dd == 7),
                            )
                for j in range(2):
                    ot = op2 * 2 + j
                    for nch in range(2):
                        y = wk.tile([128, 512], F32, tag="y", bufs=3, name="y")
                        nc.scalar.activation(
                            y[:, :], pss[j * 2 + nch][:, :],
                            mybir.ActivationFunctionType.Identity,
                            bias=bp_sb[ot][:, :],
                        )
                        nc.sync.dma_start(
                            yT[ot * 128 : (ot + 1) * 128, nch * 512 : (nch + 1) * 512],
                            y[:, :],
                        )
            wk.release()
            psp2.release()

    nc.compile()
    return nc


def kernel(x, w_qkv, b_qkv, w_proj, b_proj):
    global LAST_RESULTS
    bf = ml_dtypes.bfloat16
    x = np.asarray(x, np.float32)
    w_qkv = np.asarray(w_qkv, np.float32)
    b_qkv = np.asarray(b_qkv, np.float32)
    w_proj = np.asarray(w_proj, np.float32)
    b_proj = np.asarray(b_proj, np.float32)

    wqkvT = np.ascontiguousarray(
        np.vstack([w_qkv.T, b_qkv[None, :]]).astype(bf)
    )  # [1025, 3072]
    wprojT = np.ascontiguousarray(w_proj.T.astype(bf))  # [1024, 1024]
    bqk = np.ascontiguousarray(b_qkv[: 2 * C, None].astype(np.float32))  # [2048, 1]
    bproj = np.ascontiguousarray(b_proj[:, None].astype(np.float32))  # [1024, 1]

    in_maps = []
    xTb = {b: np.ascontiguousarray(x[b].T.astype(bf)) for b in range(B)}
    for core in range(NCORES):
        b, half = core // 2, core % 2
        own = x[b][half * NQ : (half + 1) * NQ]  # [1024, 1024]
        in_maps.append(
            {
                "xT": xTb[b],
                "xoT": np.ascontiguousarray(own.T.astype(bf)),
                "wqkvT": wqkvT,
                "bqk": bqk,
                "wprojT": wprojT,
                "bproj": bproj,
            }
        )

    if "nc" not in _CACHE:
        _CACHE["nc"] = _build()
    nc = _CACHE["nc"]

    res = run_bass_kernel_spmd(nc, in_maps, core_ids=list(range(NCORES)))
    LAST_RESULTS = res

    out = np.empty((B, N, C), np.float32)
    for core in range(NCORES):
        b, half = core // 2, core % 2
        out[b, half * NQ : (half + 1) * NQ, :] = res.results[core]["yT"].T
    return out


if __name__ == "__main__":
    rng = np.random.default_rng(0)
    s = C ** -0.5
    ins = {
        "x": rng.standard_normal((B, N, C)).astype(np.float32),
        "w_qkv": (rng.standard_normal((3 * C, C)) * s).astype(np.float32),
        "b_qkv": (rng.standard_normal(3 * C) * 0.02).astype(np.float32),
        "w_proj": (rng.standard_normal((C, C)) * s).astype(np.float32),
        "b_proj": (rng.standard_normal(C) * 0.02).astype(np.float32),
    }
    y = kernel(**ins)
    print("out", y.shape, y.dtype, float(np.abs(y).mean()))
